# revision 1
# baseline (speedup 1.0000x reference)
"""Trainium2 Bass kernel for nn_Denoiser_73598559584966.

Full-sequence self-attention (Q=K=V, no scaling) over x: [4, 16, 16, 16, 64]
  t = x.reshape(B, 4096, 64); out = softmax(t @ t^T) @ t

Sharding: 8 cores = 4 batches x 2 query-halves. Each core: 2048 queries
vs the full 4096 keys/values of its batch. No collectives.

Device algorithm per core (scores kept transposed: [keys, queries]),
key tiles processed in pairs (ktA rows 0-63 / ktB rows 64-127 of the PE
array — the second tile's weight load hides under the first's stream):
  pass1 (fp32r, contraction 64): S += k_hi . q_hi   (exact: inputs are
         pre-rounded on host to the fp32r 11-bit grid)
  pass2 (bf16, contraction 128): S += k_lo.q_hi + k_hi.q_lo - B
         (B_i = |q_i| max_j|k_j| >= rowmax -> exp never overflows; the
          shift cancels exactly in the softmax ratio)
  P = exp(S)                      ScalarE, PSUM -> fp32r SBUF
  O^T[65, q] += (V_kt|1)^T P_kt   fp32r; row 64 = softmax denominator
Device returns O^T [65, 2048]; the host epilogue divides rows 0..63 by
row 64 and transposes while gathering shards (O(N*C) marshaling).
"""
import numpy as np

B_, D_, H_, W_, C_ = 4, 16, 16, 16, 64
NTOK = D_ * H_ * W_          # 4096 tokens per batch
NQ = NTOK // 2               # 2048 queries per core
NCORES = 8
NKT = NTOK // 128            # 32 key tiles
NPAIR = NKT // 2             # 16 packed key-tile pairs
NCH = 4                      # query chunks per core
CHW = NQ // NCH              # 512 queries per chunk
NG = 4                       # DMA groups over key tiles
GKT = NKT // NG              # 8 key tiles per group

_CACHE = {}


def _round11(x):
    """Round fp32 to 11 explicit mantissa bits (fp32r grid), RNE."""
    u = np.ascontiguousarray(x, np.float32).view(np.uint32)
    bias = ((u >> 12) & 1) + np.uint32((1 << 11) - 1)
    u = (u + bias) & np.uint32(0xFFFFF000)
    return u.view(np.float32)


def _build_nc():
    import concourse.bacc as bacc
    import concourse.mybir as mybir
    from concourse.tile import TileContext

    f32 = mybir.dt.float32
    f32r = mybir.dt.float32r
    bf16 = mybir.dt.bfloat16
    EXP = mybir.ActivationFunctionType.Exp
    nc = bacc.Bacc("TRN2", target_bir_lowering=False, debug=False)

    qhh = nc.dram_tensor("qhh", [128, NQ], f32r, kind="ExternalInput")
    qp2 = nc.dram_tensor("qp2", [128, NQ], bf16, kind="ExternalInput")
    khi2 = nc.dram_tensor("khi2", [128, NTOK], f32r, kind="ExternalInput")
    kq2 = nc.dram_tensor("kq2", [128, NTOK], bf16, kind="ExternalInput")
    vpk = nc.dram_tensor("vpk", [128, NKT * 65], f32r, kind="ExternalInput")
    out = nc.dram_tensor("out", [65, NQ], f32, kind="ExternalOutput")

    GW = GKT * 128            # tokens per DMA group
    with TileContext(nc) as tc:
        with (
            tc.tile_pool(name="const", bufs=1) as const,
            tc.tile_pool(name="pp", bufs=4) as pp,
            tc.tile_pool(name="sbo", bufs=2) as sbo,
            tc.tile_pool(name="ps_s", bufs=3, space="PSUM") as ps_s,
            tc.tile_pool(name="ps_o", bufs=2, space="PSUM") as ps_o,
        ):
            # ---- PE + ACT warmup during the DMA prefix ----
            wz = const.tile([128, 512], bf16, tag="wz")
            nc.vector.memset(wz, 0.0)
            wexp = const.tile([128, 1], f32, tag="wexp")
            nc.scalar.activation(wexp, wz[:, 0:1], EXP)  # pull exp table load
            for _ in range(12):
                wps = ps_s.tile([128, 2 * CHW], f32, tag="s")
                nc.tensor.matmul(wps[:, 0:512], wz[:, 0:128], wz,
                                 start=True, stop=True)

            # ---- input DMAs (q first, then k-side in kt-groups) ----
            qhh_t = const.tile([128, NQ], f32r, tag="qhh")
            qp2_t = const.tile([128, NQ], bf16, tag="qp2")
            # chunk 0's q operands first so compute starts early
            nc.sync.dma_start(out=qhh_t[:, 0:CHW], in_=qhh[:, 0:CHW])
            nc.sync.dma_start(out=qp2_t[:, 0:CHW], in_=qp2[:, 0:CHW])
            khi2_g, kq2_g, vpk_g = [], [], []
            for g in range(NG):
                kt_ = const.tile([128, GW], f32r, tag=f"khi2_{g}")
                nc.sync.dma_start(out=kt_, in_=khi2[:, g * GW:(g + 1) * GW])
                khi2_g.append(kt_)
                kt_ = const.tile([128, GW], bf16, tag=f"kq2_{g}")
                nc.sync.dma_start(out=kt_, in_=kq2[:, g * GW:(g + 1) * GW])
                kq2_g.append(kt_)
                kt_ = const.tile([128, GKT * 65], f32r, tag=f"vpk_{g}")
                nc.sync.dma_start(
                    out=kt_, in_=vpk[:, g * GKT * 65:(g + 1) * GKT * 65])
                vpk_g.append(kt_)
                if g < NCH - 1:   # remaining q chunks, interleaved
                    cs = slice((g + 1) * CHW, (g + 2) * CHW)
                    nc.sync.dma_start(out=qhh_t[:, cs], in_=qhh[:, cs])
                    nc.sync.dma_start(out=qp2_t[:, cs], in_=qp2[:, cs])

            # ---- main loop ----
            for ch in range(NCH):
                qs = slice(ch * CHW, (ch + 1) * CHW)
                o_acc = ps_o.tile([65, CHW], f32, tag="oacc")
                for pr in range(NPAIR):
                    ktA, ktB = 2 * pr, 2 * pr + 1
                    g = ktA // GKT
                    lA = (ktA - g * GKT) * 128
                    lB = (ktB - g * GKT) * 128
                    s_t = ps_s.tile([128, 2 * CHW], f32, tag="s")
                    # pass1: k_hi . q_hi, fp32r, packed pair (rows 0-63 /
                    # 64-127) — B's weight load hides under A's stream
                    nc.tensor.matmul(
                        s_t[:, 0:CHW],
                        khi2_g[g][0:64, lA:lA + 128], qhh_t[0:64, qs],
                        start=True, stop=False,
                    )
                    nc.tensor.matmul(
                        s_t[:, CHW:2 * CHW],
                        khi2_g[g][64:128, lB:lB + 128], qhh_t[64:128, qs],
                        start=True, stop=False,
                    )
                    # pass2: cross terms + bias row, bf16, contraction 128
                    nc.tensor.matmul(
                        s_t[:, 0:CHW],
                        kq2_g[g][:, lA:lA + 128], qp2_t[:, qs],
                        start=False, stop=True,
                    )
                    nc.tensor.matmul(
                        s_t[:, CHW:2 * CHW],
                        kq2_g[g][:, lB:lB + 128], qp2_t[:, qs],
                        start=False, stop=True,
                    )
                    p_t = pp.tile([128, 2 * CHW], f32r, tag="p")
                    nc.scalar.activation(p_t, s_t, EXP)
                    for half, kt in ((0, ktA), (1, ktB)):
                        lv = (kt - g * GKT) * 65
                        nc.tensor.matmul(
                            o_acc[:, :],
                            vpk_g[g][:, lv:lv + 65],
                            p_t[:, half * CHW:(half + 1) * CHW],
                            start=(pr == 0 and half == 0),
                            stop=(pr == NPAIR - 1 and half == 1),
                            skip_group_check=True,
                        )
                # ---- ship O^T chunk (normalize + transpose on host) ----
                o_sb = sbo.tile([65, CHW], f32, tag="osb")
                nc.vector.tensor_copy(o_sb, o_acc)
                nc.sync.dma_start(out=out[:, qs], in_=o_sb)
    nc.compile()
    return nc


def _prep_inputs(x):
    """Host-side shard + operand marshaling. Returns list of 8 in_maps."""
    import ml_dtypes
    bf16 = ml_dtypes.bfloat16
    t = np.ascontiguousarray(x, np.float32).reshape(B_, NTOK, C_)
    in_maps = []
    for b in range(B_):
        kv = t[b]                                   # [4096, 64]
        k_hi = _round11(kv)
        k_lo = (kv - k_hi).astype(np.float32)
        kmax = float(np.linalg.norm(kv.astype(np.float64), axis=1).max())
        khi2 = np.concatenate([k_hi.T, k_hi.T]).astype(np.float32)
        kq2 = np.concatenate(
            [k_lo.T[0:63], np.ones((1, NTOK), np.float32), k_hi.T]
        ).astype(bf16)
        vpk = np.concatenate(
            [np.concatenate([kv[i * 128:(i + 1) * 128],
                             np.ones((128, 1), np.float32)], axis=1)
             for i in range(NKT)], axis=1).astype(np.float32)  # [128, 32*65]
        for h in range(2):
            q = t[b, h * NQ:(h + 1) * NQ]           # [2048, 64]
            q_hi = _round11(q)
            q_lo = (q - q_hi).astype(np.float32)
            qn = np.linalg.norm(q.astype(np.float64), axis=1)
            bias = (qn * kmax + 0.125).astype(np.float32)   # >= rowmax(s)
            qhh = np.concatenate([q_hi.T, q_hi.T]).astype(np.float32)
            qp2 = np.concatenate(
                [q_hi.T[0:63], -bias[None, :], q_lo.T]).astype(bf16)
            in_maps.append({
                "qhh": qhh, "qp2": qp2, "khi2": khi2, "kq2": kq2, "vpk": vpk,
            })
    return in_maps


def run(x, trace=False):
    from concourse.bass_utils import run_bass_kernel_spmd
    if "nc" not in _CACHE:
        _CACHE["nc"] = _build_nc()
    nc = _CACHE["nc"]
    in_maps = _prep_inputs(x)
    res = run_bass_kernel_spmd(
        nc, in_maps, core_ids=list(range(NCORES)), trace=trace,
    )
    full = np.empty((B_, NTOK, C_), np.float32)
    for b in range(B_):
        for h in range(2):
            o = res.results[2 * b + h]["out"]        # [65, 2048]
            full[b, h * NQ:(h + 1) * NQ] = (o[0:C_] / o[C_]).T
    return full.reshape(B_, D_, H_, W_, C_), res


def kernel(x):
    out, _ = run(x, trace=False)
    return out



# revision 4
# speedup vs baseline: 1.4540x; 1.4540x over previous
"""Trainium2 Bass kernel for nn_Denoiser_73598559584966.

Full-sequence self-attention (Q=K=V, no scaling) over x: [4, 16, 16, 16, 64]
  t = x.reshape(B, 4096, 64); out = softmax(t @ t^T) @ t
Sharding: 8 cores = 4 batches x 2 query-halves. Each core: 2048 queries
vs the full 4096 keys/values of its batch. No collectives.

v2 device algorithm (single-pass bf16 scores; scores kept transposed
[keys, queries]):
  S'' = (K|1)^T (Q | 88.0-bias)    one bf16 matmul per (key-tile, chunk),
        contraction 65 = 64 channels + per-query bias row. bias_i ~ |q_i|^2
        so p_max ~ 1; the +88.0 pre-biases for the schraudolph path.
  P = exp(S'' - 88.0)              alternating per key tile:
        even kt -> ScalarE ACT (exact exp, bias immediate = -88.0)
        odd  kt -> VectorE one tensor_scalar: i16 = max(S''*(128/ln2), 0),
                   whose int16 bits reinterpreted as bf16 ARE exp(S''-88)
                   to ~3% (Schraudolph); clamp makes negatives exact 0.
  O^T[65, q] += (V_kt|1)^T P_kt    bf16, accumulated in PSUM; row 64 = sum(P)
Host epilogue divides rows 0..63 by row 64 and transposes while gathering.

Queries processed as 2 chunk-pairs of (512|512) columns sharing each
weight load; exp granularity [128, 1024] (one PSUM bank-pair).
"""
import math
import numpy as np

B_, D_, H_, W_, C_ = 4, 16, 16, 16, 64
NTOK = D_ * H_ * W_          # 4096 tokens per batch
NQ = NTOK // 2               # 2048 queries per core
NCORES = 8
NKT = NTOK // 128            # 32 key tiles
CHW = 512                    # queries per chunk (PSUM bank width)
NCP = 2                      # chunk-pairs (1024 queries each)
NG = 4                       # DMA groups over key tiles
GKT = NKT // NG              # 8 key tiles per group

# Schraudolph constants in bf16-bit space.
A_EXP = 128.0 / math.log(2.0)                       # 184.665
SHIFT = (16256.0 - 366393.0 / 65536.0) / A_EXP      # 87.99942

_CACHE = {}


def _build_nc():
    import concourse.bacc as bacc
    import concourse.mybir as mybir
    from concourse.tile import TileContext

    f32 = mybir.dt.float32
    bf16 = mybir.dt.bfloat16
    i16 = mybir.dt.int16
    EXP = mybir.ActivationFunctionType.Exp
    MULT = mybir.AluOpType.mult
    MAX = mybir.AluOpType.max
    nc = bacc.Bacc("TRN2", target_bir_lowering=False, debug=False)

    kq = nc.dram_tensor("kq", [65, NTOK], bf16, kind="ExternalInput")
    qb = nc.dram_tensor("qb", [65, NQ], bf16, kind="ExternalInput")
    vpk = nc.dram_tensor("vpk", [128, NKT * 65], bf16, kind="ExternalInput")
    out = nc.dram_tensor("out", [65, NQ], f32, kind="ExternalOutput")

    GW = GKT * 128            # tokens per kq DMA group
    with TileContext(nc) as tc:
        with (
            tc.tile_pool(name="const", bufs=1) as const,
            tc.tile_pool(name="pp", bufs=4) as pp,
            tc.tile_pool(name="sbo", bufs=2) as sbo,
            tc.tile_pool(name="ps_s", bufs=3, space="PSUM") as ps_s,
            tc.tile_pool(name="ps_o", bufs=1, space="PSUM") as ps_o,
        ):
            # ---- PE + ACT warmup during the DMA prefix ----
            wz = const.tile([128, 512], bf16, tag="wz")
            nc.vector.memset(wz, 0.0)
            bshift = const.tile([128, 1], f32, tag="bshift")
            nc.vector.memset(bshift, -SHIFT)
            wexp = const.tile([128, 1], f32, tag="wexp")
            nc.scalar.activation(wexp, wz[:, 0:1], EXP)  # pull exp table load
            for _ in range(12):
                wps = ps_s.tile([128, 2 * CHW], f32, tag="s")
                nc.tensor.matmul(wps[:, 0:512], wz[:, 0:128], wz,
                                 start=True, stop=True)

            # ---- input DMAs ----
            qb_t = const.tile([65, NQ], bf16, tag="qb")
            kq_t = const.tile([65, NTOK], bf16, tag="kq")
            vpk_t = const.tile([128, NKT * 65], bf16, tag="vpk")
            nc.sync.dma_start(out=qb_t[:, 0:1024], in_=qb[:, 0:1024])
            for g in range(NG):
                nc.sync.dma_start(out=kq_t[:, g * GW:(g + 1) * GW],
                                  in_=kq[:, g * GW:(g + 1) * GW])
                nc.sync.dma_start(
                    out=vpk_t[:, g * GKT * 65:(g + 1) * GKT * 65],
                    in_=vpk[:, g * GKT * 65:(g + 1) * GKT * 65])
                if g == 0:
                    nc.sync.dma_start(out=qb_t[:, 1024:2048],
                                      in_=qb[:, 1024:2048])

            # ---- main loop: 2 chunk-pairs x 32 key tiles ----
            for cp in range(NCP):
                q0 = slice(cp * 1024, cp * 1024 + 512)
                q1 = slice(cp * 1024 + 512, cp * 1024 + 1024)
                o0 = ps_o.tile([65, CHW], f32, tag="o0")
                o1 = ps_o.tile([65, CHW], f32, tag="o1")
                for kt in range(NKT):
                    ks = slice(kt * 128, (kt + 1) * 128)
                    s_t = ps_s.tile([128, 2 * CHW], f32, tag="s")
                    nc.tensor.matmul(s_t[:, 0:CHW], kq_t[:, ks], qb_t[:, q0],
                                     start=True, stop=True)
                    nc.tensor.matmul(s_t[:, CHW:2 * CHW], kq_t[:, ks],
                                     qb_t[:, q1], start=True, stop=True)
                    p_t = pp.tile([128, 2 * CHW], bf16, tag="p")
                    if kt % 2 == 0:
                        nc.scalar.activation(p_t, s_t, EXP, bias=bshift)
                    else:
                        nc.vector.tensor_scalar(
                            p_t.bitcast(i16), s_t, A_EXP, 0.0, MULT, MAX)
                    vs = slice(kt * 65, kt * 65 + 65)
                    nc.tensor.matmul(
                        o0, vpk_t[:, vs], p_t[:, 0:CHW],
                        start=(kt == 0), stop=(kt == NKT - 1),
                        skip_group_check=True)
                    nc.tensor.matmul(
                        o1, vpk_t[:, vs], p_t[:, CHW:2 * CHW],
                        start=(kt == 0), stop=(kt == NKT - 1),
                        skip_group_check=True)
                # ---- ship O^T (normalize + transpose on host) ----
                o0sb = sbo.tile([65, CHW], f32, tag="o0sb")
                nc.scalar.copy(o0sb, o0)
                nc.sync.dma_start(out=out[:, q0], in_=o0sb)
                o1sb = sbo.tile([65, CHW], f32, tag="o1sb")
                nc.vector.tensor_copy(o1sb, o1)
                nc.sync.dma_start(out=out[:, q1], in_=o1sb)
    nc.compile()
    return nc


def _prep_inputs(x):
    """Host-side shard + operand marshaling. Returns list of 8 in_maps."""
    import ml_dtypes
    bf16 = ml_dtypes.bfloat16
    t = np.ascontiguousarray(x, np.float32).reshape(B_, NTOK, C_)
    in_maps = []
    for b in range(B_):
        kv = t[b]                                   # [4096, 64]
        kmax = float(np.linalg.norm(kv.astype(np.float64), axis=1).max())
        kq = np.concatenate(
            [kv.T, np.ones((1, NTOK), np.float32)]).astype(bf16)
        vpk = np.concatenate(
            [np.concatenate([kv[i * 128:(i + 1) * 128],
                             np.ones((128, 1), np.float32)], axis=1)
             for i in range(NKT)], axis=1).astype(bf16)   # [128, 32*65]
        for h in range(2):
            q = t[b, h * NQ:(h + 1) * NQ]           # [2048, 64]
            qn = np.linalg.norm(q.astype(np.float64), axis=1)
            bias = np.minimum(
                np.maximum(qn * qn + 0.5, qn * kmax - 80.0), 130.0)
            brow = (np.float32(SHIFT) - bias.astype(np.float32))
            qb = np.concatenate([q.T, brow[None, :]]).astype(bf16)
            in_maps.append({"kq": kq, "qb": qb, "vpk": vpk})
    return in_maps


def run(x, trace=False):
    from concourse.bass_utils import run_bass_kernel_spmd
    if "nc" not in _CACHE:
        _CACHE["nc"] = _build_nc()
    nc = _CACHE["nc"]
    in_maps = _prep_inputs(x)
    res = run_bass_kernel_spmd(
        nc, in_maps, core_ids=list(range(NCORES)), trace=trace,
    )
    full = np.empty((B_, NTOK, C_), np.float32)
    for b in range(B_):
        for h in range(2):
            o = res.results[2 * b + h]["out"]        # [65, 2048]
            full[b, h * NQ:(h + 1) * NQ] = (o[0:C_] / o[C_]).T
    return full.reshape(B_, D_, H_, W_, C_), res


def kernel(x):
    out, _ = run(x, trace=False)
    return out


# revision 5
# speedup vs baseline: 1.4703x; 1.0112x over previous
"""Trainium2 Bass kernel for nn_Denoiser_73598559584966.

Full-sequence self-attention (Q=K=V, no scaling) over x: [4, 16, 16, 16, 64]
  t = x.reshape(B, 4096, 64); out = softmax(t @ t^T) @ t
Sharding: 8 cores = 4 batches x 2 query-halves. Each core: 2048 queries
vs the full 4096 keys/values of its batch. No collectives.

v3 device algorithm (single-pass bf16 scores; scores kept transposed
[keys, queries]; two decoupled 512-query half-pipelines per key tile):
  S'' = (K|1)^T (Q | 88.0-bias)    one bf16 matmul per (key-tile, half),
        contraction 65 = 64 channels + per-query bias row. bias_i ~ |q_i|^2
        so p_max ~ 1; the +88.0 pre-biases for the schraudolph path.
  P = exp(S'' - 88.0)              half 0 -> ScalarE ACT (exact exp,
        per-partition bias AP = -88.0); half 1 -> VectorE single
        tensor_scalar: i16 = max(S''*(128/ln2), 0), whose int16 bits
        reinterpreted as bf16 ARE exp(S''-88) to ~3% (Schraudolph);
        the max-0 clamp maps underflow to +0.0 exactly.
  O^T[65, q] += (V_kt|1)^T P_kt    bf16, accumulated in PSUM; row 64 = sum(P)
Host epilogue divides rows 0..63 by row 64 and transposes while gathering.
"""
import math
import numpy as np

B_, D_, H_, W_, C_ = 4, 16, 16, 16, 64
NTOK = D_ * H_ * W_          # 4096 tokens per batch
NQ = NTOK // 2               # 2048 queries per core
NCORES = 8
NKT = NTOK // 128            # 32 key tiles
CHW = 512                    # queries per chunk (PSUM bank width)
NCP = 2                      # chunk-pairs (1024 queries each)
NG = 4                       # DMA groups over key tiles
GKT = NKT // NG              # 8 key tiles per group

# Schraudolph constants in bf16-bit space.
A_EXP = 128.0 / math.log(2.0)                       # 184.665
SHIFT = (16256.0 - 366393.0 / 65536.0) / A_EXP      # 87.99942

_CACHE = {}


def _build_nc():
    import concourse.bacc as bacc
    import concourse.mybir as mybir
    from concourse.tile import TileContext

    f32 = mybir.dt.float32
    bf16 = mybir.dt.bfloat16
    i16 = mybir.dt.int16
    EXP = mybir.ActivationFunctionType.Exp
    MULT = mybir.AluOpType.mult
    MAX = mybir.AluOpType.max
    nc = bacc.Bacc("TRN2", target_bir_lowering=False, debug=False)

    kq = nc.dram_tensor("kq", [65, NTOK], bf16, kind="ExternalInput")
    qb = nc.dram_tensor("qb", [65, NQ], bf16, kind="ExternalInput")
    vpk = nc.dram_tensor("vpk", [128, NKT * 65], bf16, kind="ExternalInput")
    out = nc.dram_tensor("out", [65, NQ], bf16, kind="ExternalOutput")

    GW = GKT * 128            # tokens per kq DMA group
    with TileContext(nc) as tc:
        with (
            tc.tile_pool(name="const", bufs=1) as const,
            tc.tile_pool(name="pp", bufs=4) as pp,
            tc.tile_pool(name="sbo", bufs=2) as sbo,
            tc.tile_pool(name="ps_s", bufs=3, space="PSUM") as ps_s,
            tc.tile_pool(name="ps_o", bufs=1, space="PSUM") as ps_o,
        ):
            # ---- PE + ACT warmup during the DMA prefix ----
            wz = const.tile([128, 512], bf16, tag="wz")
            nc.gpsimd.memset(wz, 0.0)
            bshift = const.tile([128, 1], f32, tag="bshift")
            nc.vector.memset(bshift, -SHIFT)
            wexp = const.tile([128, 1], f32, tag="wexp")
            nc.scalar.activation(wexp, wz[:, 0:1], EXP)  # pull exp table load
            for i in range(8):
                wps = ps_s.tile([128, CHW], f32, tag="s0" if i % 2 == 0 else "s1")
                nc.tensor.matmul(wps, wz[:, 0:128], wz,
                                 start=True, stop=True)

            # ---- input DMAs ----
            qb_t = const.tile([65, NQ], bf16, tag="qb")
            kq_t = const.tile([65, NTOK], bf16, tag="kq")
            vpk_t = const.tile([128, NKT * 65], bf16, tag="vpk")
            nc.sync.dma_start(out=qb_t[:, 0:1024], in_=qb[:, 0:1024])
            for g in range(NG):
                nc.sync.dma_start(out=kq_t[:, g * GW:(g + 1) * GW],
                                  in_=kq[:, g * GW:(g + 1) * GW])
                nc.sync.dma_start(
                    out=vpk_t[:, g * GKT * 65:(g + 1) * GKT * 65],
                    in_=vpk[:, g * GKT * 65:(g + 1) * GKT * 65])
                if g == 0:
                    nc.sync.dma_start(out=qb_t[:, 1024:2048],
                                      in_=qb[:, 1024:2048])

            # ---- main loop: 2 chunk-pairs x 32 key tiles ----
            for cp in range(NCP):
                q0 = slice(cp * 1024, cp * 1024 + 512)
                q1 = slice(cp * 1024 + 512, cp * 1024 + 1024)
                o0 = ps_o.tile([65, CHW], f32, tag="o0")
                o1 = ps_o.tile([65, CHW], f32, tag="o1")
                for kt in range(NKT):
                    ks = slice(kt * 128, (kt + 1) * 128)
                    vs = slice(kt * 65, kt * 65 + 65)
                    s0 = ps_s.tile([128, CHW], f32, tag="s0")
                    s1 = ps_s.tile([128, CHW], f32, tag="s1")
                    nc.tensor.matmul(s0, kq_t[:, ks], qb_t[:, q0],
                                     start=True, stop=True)
                    nc.tensor.matmul(s1, kq_t[:, ks], qb_t[:, q1],
                                     start=True, stop=True)
                    p0 = pp.tile([128, CHW], bf16, tag="p0")
                    p1 = pp.tile([128, CHW], bf16, tag="p1")
                    nc.scalar.activation(p0, s0, EXP, bias=bshift)
                    nc.vector.tensor_scalar(
                        p1.bitcast(i16), s1, A_EXP, 0.0, MULT, MAX)
                    nc.tensor.matmul(
                        o0, vpk_t[:, vs], p0,
                        start=(kt == 0), stop=(kt == NKT - 1),
                        skip_group_check=True)
                    nc.tensor.matmul(
                        o1, vpk_t[:, vs], p1,
                        start=(kt == 0), stop=(kt == NKT - 1),
                        skip_group_check=True)
                # ---- ship O^T (normalize + transpose on host) ----
                o0sb = sbo.tile([65, CHW], bf16, tag="o0sb")
                nc.scalar.copy(o0sb, o0)
                nc.sync.dma_start(out=out[:, q0], in_=o0sb)
                o1sb = sbo.tile([65, CHW], bf16, tag="o1sb")
                nc.vector.tensor_copy(o1sb, o1)
                nc.sync.dma_start(out=out[:, q1], in_=o1sb)
    nc.compile()
    return nc


def _prep_inputs(x):
    """Host-side shard + operand marshaling. Returns list of 8 in_maps."""
    import ml_dtypes
    bf16 = ml_dtypes.bfloat16
    t = np.ascontiguousarray(x, np.float32).reshape(B_, NTOK, C_)
    in_maps = []
    for b in range(B_):
        kv = t[b]                                   # [4096, 64]
        kmax = float(np.linalg.norm(kv.astype(np.float64), axis=1).max())
        kq = np.concatenate(
            [kv.T, np.ones((1, NTOK), np.float32)]).astype(bf16)
        vpk = np.concatenate(
            [np.concatenate([kv[i * 128:(i + 1) * 128],
                             np.ones((128, 1), np.float32)], axis=1)
             for i in range(NKT)], axis=1).astype(bf16)   # [128, 32*65]
        for h in range(2):
            q = t[b, h * NQ:(h + 1) * NQ]           # [2048, 64]
            qn = np.linalg.norm(q.astype(np.float64), axis=1)
            bias = np.minimum(
                np.maximum(qn * qn + 0.5, qn * kmax - 80.0), 130.0)
            brow = (np.float32(SHIFT) - bias.astype(np.float32))
            qb = np.concatenate([q.T, brow[None, :]]).astype(bf16)
            in_maps.append({"kq": kq, "qb": qb, "vpk": vpk})
    return in_maps


def run(x, trace=False):
    from concourse.bass_utils import run_bass_kernel_spmd
    if "nc" not in _CACHE:
        _CACHE["nc"] = _build_nc()
    nc = _CACHE["nc"]
    in_maps = _prep_inputs(x)
    res = run_bass_kernel_spmd(
        nc, in_maps, core_ids=list(range(NCORES)), trace=trace,
    )
    full = np.empty((B_, NTOK, C_), np.float32)
    for b in range(B_):
        for h in range(2):
            o = res.results[2 * b + h]["out"].astype(np.float32)  # [65, 2048]
            full[b, h * NQ:(h + 1) * NQ] = (o[0:C_] / o[C_]).T
    return full.reshape(B_, D_, H_, W_, C_), res


def kernel(x):
    out, _ = run(x, trace=False)
    return out
